# revision 16
# baseline (speedup 1.0000x reference)
"""GQA sliding-window attention (B=1, T=2048, C=2048, 32 Q / 8 KV heads,
head_dim=64, window=512, 16 global tokens) on 8 Trainium2 NeuronCores.

Sharding: tensor-parallel over heads — core c owns KV head c and Q heads
4c..4c+3.  Everything on-device runs transposed ([feature, token] layout).

v2 layout: x/weights/gathered-y all bf16 (rel-err budget 2e-2, measured
~1e-3); per-(head,chunk) scores are packed into one 5-bank PSUM strip
(cols 0-1279 = window lower-edge blocks, 1280-2559 = causal diag blocks)
so softmax exp is 2 big activations instead of 8 small ones; 1/den via
vector reciprocal (no Exp/Ln act-table thrash); masks split between
gpsimd affine_select (edge) and vector mul (diag); PV lags scores by one
head so the PE never waits on exp.  yT is AllGather'd per 512-token
chunk in bf16 (overlapped with the next chunk's attention); WO runs two
chunks behind.
"""

import sys
sys.path.insert(0, "/opt/trn_rl_repo")

import numpy as np

import concourse.bass as bass
import concourse.mybir as mybir
from concourse import bacc
from concourse.tile import TileContext
from concourse.masks import make_identity

f32 = mybir.dt.float32
f32r = mybir.dt.float32r
bf16 = mybir.dt.bfloat16
AF = mybir.ActivationFunctionType

N_CORES = 8
T = 2048
C = 2048
HD = 64
NH_LOC = 4            # query heads per core
QD = NH_LOC * HD      # 256 per-core query dims
NB = T // 128         # 16 token blocks
NSB = T // 512        # 4 superblocks
N_GLOBAL = 16
SCALE = 0.125         # 1/sqrt(64)

_CACHE = {}


def _segs(Q, which):
    """Score-strip segment tables: (j, col0, qlo, nblk).

    strip1 (cols 0-1279): window lower-edge k-blocks j<4Q, masked block is
    the last 128-col block of each segment (q-block j+4).
    strip2 (cols 1280-2559): causal k-blocks j>=4Q, masked block is the
    first 128-col block (q-block j).
    Column packing keeps every matmul region inside one 2KB psum bank.
    """
    if which == 1:
        if Q == 0:
            return []
        return [(4 * Q - 1, 0, 4 * Q, 4), (4 * Q - 2, 512, 4 * Q, 3),
                (4 * Q - 4, 896, 4 * Q, 1), (4 * Q - 3, 1024, 4 * Q, 2)]
    return [(4 * Q + 2, 1280, 4 * Q + 2, 2), (4 * Q, 1536, 4 * Q, 4),
            (4 * Q + 1, 2048, 4 * Q + 1, 3), (4 * Q + 3, 2432, 4 * Q + 3, 1)]


def _build():
    nc = bacc.Bacc(num_devices=N_CORES)

    xT = nc.declare_dram_parameter("xT", [C, T], bf16, isOutput=False)
    wqkvT = nc.declare_dram_parameter("wqkvT", [C, QD + 128], bf16,
                                      isOutput=False)
    woT = nc.declare_dram_parameter("woT", [C, QD], bf16, isOutput=False)
    cs1 = nc.declare_dram_parameter("cs128", [128, T], f32, isOutput=False)
    sn1 = nc.declare_dram_parameter("sn128", [128, T], f32, isOutput=False)
    outT = nc.declare_dram_parameter("outT", [QD, T], f32, isOutput=True)

    with TileContext(nc) as tc:
        with tc.tile_pool(name="persist", bufs=1) as pp, \
             tc.tile_pool(name="psAll", bufs=1, space="PSUM") as psA, \
             tc.tile_pool(name="pdram", bufs=1, space="DRAM") as pdr:
            # ---- persistent state ----
            qTr01 = pp.tile([128, T], f32r)   # rotated q, heads 0,1
            qTr23 = pp.tile([128, T], f32r)   # rotated q, heads 2,3
            kTr2 = pp.tile([128, T], f32r)    # rotated k duplicated on halves
            vT = pp.tile([64, T], f32)        # v (d, t) pre-transpose
            vgs = [pp.tile([128, HD + 1], f32r, name=f"vg{j}", tag=f"vg{j}")
                   for j in range(NB)]
            ytn = [pp.tile([64, T], bf16, name=f"ytn{h}", tag=f"ytn{h}")
                   for h in range(NH_LOC)]
            csb = pp.tile([128, T], f32)
            snb = pp.tile([128, T], f32)
            ident = pp.tile([128, 128], f32)
            m_diag = pp.tile([128, 128], f32)

            make_identity(nc, ident[:])
            # m_diag keeps p <= f (causal; scoresT layout: partition=key,
            # free=query)
            nc.gpsimd.memset(m_diag[:], 1.0)
            nc.gpsimd.affine_select(out=m_diag[:], in_=m_diag[:],
                                    compare_op=mybir.AluOpType.is_ge, fill=0.0,
                                    base=0, pattern=[[1, 128]],
                                    channel_multiplier=-1)
            ones1 = pp.tile([128, 1], f32)
            nc.vector.memset(ones1[:], 1.0)
            for j in range(NB):
                nc.vector.tensor_copy(vgs[j][:, HD:HD + 1], ones1[:])

            # tiny warmup AllGather so the first real one doesn't pay the
            # collective-stream cold start; overlaps the QKV phase
            agw_i = pdr.tile([1, 64], f32, name="agw_i", tag="agw_i")
            agw_o = pdr.tile([N_CORES, 64], f32, name="agw_o", tag="agw_o",
                             addr_space="Shared")
            nc.gpsimd.collective_compute(
                "AllGather", mybir.AluOpType.bypass,
                replica_groups=[list(range(N_CORES))],
                ins=[agw_i[:]], outs=[agw_o[:]])

            # psum tags (8 banks): ss = score strip / phase-A projections
            # (5 banks); yq = PV accumulator + WO psum (2 banks); aux =
            # V-transpose + global-score scratch (1 bank)
            def ps_ss(name):
                return psA.tile([128, 2560], f32, tag="ss", name=name,
                                padded_shape=[128, 2560])

            def ps_yq(name):
                return psA.tile([128, 512], f32, tag="yq", bufs=2, name=name,
                                padded_shape=[128, 512])

            def ps_aux(name):
                return psA.tile([128, 512], f32, tag="aux", bufs=1, name=name,
                                padded_shape=[128, 512])

            with tc.tile_pool(name="pbc", bufs=1) as pbc:
                # ============= phase A: QKV projections + RoPE =============
                with tc.tile_pool(name="pa", bufs=1) as pa:
                    wqkv_sb = [pa.tile([128, QD + 128], bf16, name=f"wqkv{k}",
                                       tag=f"wqkv{k}") for k in range(16)]
                    xt_sb = [pa.tile([128, T], bf16, name=f"xt{k}",
                                     tag=f"xt{k}") for k in range(16)]
                    for kt in range(16):
                        nc.sync.dma_start(
                            out=wqkv_sb[kt][:],
                            in_=wqkvT[128 * kt:128 * (kt + 1), :])
                        nc.sync.dma_start(
                            out=xt_sb[kt][:, 0:1024],
                            in_=xT[128 * kt:128 * (kt + 1), 0:1024])
                        if kt == 4:
                            nc.sync.dma_start(out=csb[:], in_=cs1[:])
                            nc.sync.dma_start(out=snb[:], in_=sn1[:])
                    for kt in range(16):
                        nc.sync.dma_start(
                            out=xt_sb[kt][:, 1024:2048],
                            in_=xT[128 * kt:128 * (kt + 1), 1024:2048])

                    def rope(dst, psrc, rows, col0):
                        """dst[0:rows, col0:+512] = rope(psrc); head dims are
                        pre-permuted to rotate-half order (evens then odds).
                        Swap-copies split across scalar+vector so neither
                        engine paces the chunk."""
                        qs = pa.tile([rows, 512], f32, tag="qs", bufs=2)
                        for b in range(rows // 32):
                            s = b ^ 1
                            eng = nc.scalar.copy if b % 2 == 0 else \
                                nc.vector.tensor_copy
                            eng(qs[32 * b:32 * b + 32, :],
                                psrc[32 * s:32 * s + 32, :])
                        t1 = pa.tile([rows, 512], f32, tag="t1", bufs=2)
                        t2 = pa.tile([rows, 512], f32, tag="t2", bufs=2)
                        nc.vector.tensor_mul(t1[:], psrc,
                                             csb[0:rows, col0:col0 + 512])
                        nc.vector.tensor_mul(t2[:], qs[:],
                                             snb[0:rows, col0:col0 + 512])
                        nc.vector.tensor_add(dst[0:rows, col0:col0 + 512],
                                             t1[:], t2[:])

                    # one strip for all of phase A; q01/q23 double-buffer by
                    # alternating column slots (subtile deps), kv alternates
                    # strip slot 4 / a yq-tag bank
                    ssA = ps_ss("ssA")
                    for tcc in range(NSB):
                        c0 = 512 * tcc
                        od = tcc % 2
                        q01 = ssA[0:128, 512 * od:512 * od + 512]
                        q23 = ssA[0:128, 1024 + 512 * od:1536 + 512 * od]
                        if od == 0:
                            kv = ssA[0:128, 2048:2560]
                            kv_k = ssA[0:64, 2048:2560]
                            kv_v = ssA[64:128, 2048:2560]
                        else:
                            kvt = ps_yq(f"kv{tcc}")
                            kv = kvt[0:128, 0:512]
                            kv_k = kvt[0:64, 0:512]
                            kv_v = kvt[64:128, 0:512]
                        # region-sequential matmuls so each region's rope
                        # overlaps the next region's matmuls
                        for kt in range(16):
                            nc.tensor.matmul(q01, wqkv_sb[kt][:, 0:128],
                                             xt_sb[kt][:, c0:c0 + 512],
                                             start=kt == 0, stop=kt == 15)
                        rope(qTr01, q01, 128, c0)
                        for kt in range(16):
                            nc.tensor.matmul(q23, wqkv_sb[kt][:, 128:256],
                                             xt_sb[kt][:, c0:c0 + 512],
                                             start=kt == 0, stop=kt == 15)
                        rope(qTr23, q23, 128, c0)
                        for kt in range(16):
                            nc.tensor.matmul(kv, wqkv_sb[kt][:, 256:384],
                                             xt_sb[kt][:, c0:c0 + 512],
                                             start=kt == 0, stop=kt == 15)
                        rope(kTr2, kv_k, 64, c0)
                        for b in range(2):
                            nc.vector.tensor_copy(
                                kTr2[64 + 32 * b:96 + 32 * b, c0:c0 + 512],
                                kTr2[32 * b:32 * (b + 1), c0:c0 + 512])
                        nc.scalar.copy(vT[:, c0:c0 + 512], kv_v)
                        for j in range(4 * tcc, 4 * tcc + 4):
                            ptr = ps_aux(f"ptr{j}")
                            nc.tensor.transpose(ptr[0:128, 0:64],
                                                vT[:, 128 * j:128 * (j + 1)],
                                                ident[0:64, 0:64])
                            nc.scalar.copy(vgs[j][:, 0:HD], ptr[0:128, 0:64])

                # ===== attention (Q-outer) + chunked AllGather + lagged WO ==
                with tc.tile_pool(name="pc", bufs=1) as pc:
                    wo_sb = [pc.tile([128, QD], bf16, name=f"wo{k}",
                                     tag=f"wo{k}") for k in range(16)]
                    for k in range(16):
                        nc.sync.dma_start(out=wo_sb[k][:],
                                          in_=woT[128 * k:128 * (k + 1), :])
                    agi = [pdr.tile([QD, 512], bf16, name=f"agi{Q}",
                                    tag=f"agi{Q}") for Q in range(NSB)]
                    ago = [pdr.tile([C, 512], bf16, name=f"ago{Q}",
                                    tag=f"ago{Q}", addr_space="Shared")
                           for Q in range(NSB)]

                    def attn_scores(h, Q, ss):
                        """Emit score matmuls + exps + masks for head h of
                        chunk Q into the shared psum strip; returns the state
                        the (lagged) PV stage needs."""
                        qt = qTr01 if h < 2 else qTr23
                        qb = 64 * (h % 2)
                        c0 = 512 * Q
                        sg = ps_aux(f"sg{Q}_{h}")
                        nc.tensor.matmul(sg[0:N_GLOBAL, 0:512],
                                         kTr2[qb:qb + 64, 0:N_GLOBAL],
                                         qt[qb:qb + 64, c0:c0 + 512],
                                         start=True, stop=True)
                        pg = pbc.tile([N_GLOBAL, 512], f32r, tag="pg", bufs=3,
                                      name=f"pg{Q}_{h}")
                        nc.scalar.activation(pg[:], sg[0:N_GLOBAL, 0:512],
                                             AF.Exp, scale=SCALE)
                        pt = pbc.tile([128, 2560], f32r, tag="pt", bufs=3,
                                      name=f"pt{Q}_{h}")
                        s1, s2 = _segs(Q, 1), _segs(Q, 2)
                        for j, a, qlo, nblk in s1 + s2:
                            nc.tensor.matmul(
                                ss[0:128, a:a + 128 * nblk],
                                kTr2[qb:qb + 64, 128 * j:128 * (j + 1)],
                                qt[qb:qb + 64,
                                   128 * qlo:128 * qlo + 128 * nblk],
                                start=True, stop=True)
                        if s1:
                            nc.scalar.activation(pt[:, 0:1280],
                                                 ss[0:128, 0:1280],
                                                 AF.Exp, scale=SCALE)
                            for j, a, qlo, nblk in s1:
                                e = a + 128 * (j + 4 - qlo)
                                nc.gpsimd.affine_select(
                                    out=pt[:, e:e + 128], in_=pt[:, e:e + 128],
                                    compare_op=mybir.AluOpType.is_ge,
                                    fill=0.0, base=-1, pattern=[[-1, 128]],
                                    channel_multiplier=1)
                        nc.scalar.activation(pt[:, 1280:2560],
                                             ss[0:128, 1280:2560],
                                             AF.Exp, scale=SCALE)
                        for j, a, qlo, nblk in s2:
                            nc.vector.tensor_mul(pt[:, a:a + 128],
                                                 pt[:, a:a + 128], m_diag[:])
                        # kill window copies of global keys in k-block 0
                        if Q <= 1:
                            for j, a, qlo, nblk in (s1 + s2):
                                if j == 0:
                                    nc.gpsimd.affine_select(
                                        out=pt[:, a:a + 128 * nblk],
                                        in_=pt[:, a:a + 128 * nblk],
                                        compare_op=mybir.AluOpType.is_ge,
                                        fill=0.0, base=-N_GLOBAL,
                                        pattern=[[0, 128 * nblk]],
                                        channel_multiplier=1)
                        return (h, Q, pt, pg)

                    def attn_pv(state):
                        """Lagged PV + normalize + ship to the gather buf."""
                        h, Q, pt, pg = state
                        c0 = 512 * Q
                        yq = ps_yq(f"yq{Q}_{h}")
                        nc.tensor.matmul(yq[0:HD + 1, 0:512],
                                         vgs[0][0:N_GLOBAL, :], pg[:],
                                         start=True, stop=False)
                        segs = _segs(Q, 1) + _segs(Q, 2)
                        for idx, (j, a, qlo, nblk) in enumerate(segs):
                            o = 128 * (qlo - 4 * Q)
                            nc.tensor.matmul(
                                yq[0:HD + 1, o:o + 128 * nblk],
                                vgs[j][:], pt[:, a:a + 128 * nblk],
                                start=False, stop=(idx == len(segs) - 1))
                        dens = pbc.tile([1, 512], f32, tag="dens", bufs=2,
                                        name=f"dens{Q}_{h}")
                        nc.vector.tensor_copy(dens[:], yq[HD:HD + 1, 0:512])
                        denr = pbc.tile([1, 512], f32, tag="denr", bufs=2,
                                        name=f"denr{Q}_{h}")
                        nc.vector.reciprocal_approx_fast(denr[:], dens[:])
                        rb = pbc.tile([64, 512], f32, tag="rb", bufs=2,
                                      name=f"rb{Q}_{h}")
                        nc.gpsimd.partition_broadcast(rb[:], denr[:])
                        nc.vector.tensor_mul(ytn[h][:, c0:c0 + 512],
                                             yq[0:HD, 0:512], rb[:])
                        nc.sync.dma_start(
                            out=agi[Q][64 * h:64 * (h + 1), :],
                            in_=ytn[h][:, c0:c0 + 512])

                    def wo_load(Q):
                        yt = pc.tile([128, 16 * 512], bf16, tag="yt", bufs=2,
                                     name=f"yt{Q}")
                        # yt[p, 512*ci + c] = ago[Q][128*ci + p, c]
                        nc.sync.dma_start(
                            out=yt.rearrange("p (ci c) -> p ci c", c=512),
                            in_=ago[Q].rearrange("(ci p) c -> p ci c", p=128))
                        return yt

                    def wo_chunk(Q, yt):
                        c0 = 512 * Q
                        for ob in range(2):
                            wp = ps_yq(f"wp{Q}_{ob}")
                            for ci in range(16):
                                nc.tensor.matmul(
                                    wp[0:128, 0:512],
                                    wo_sb[ci][:, 128 * ob:128 * (ob + 1)],
                                    yt[:, 512 * ci:512 * (ci + 1)],
                                    start=(ci == 0), stop=(ci == 15))
                            ot = pc.tile([128, 512], f32, tag="ot", bufs=2,
                                         name=f"ot{Q}_{ob}")
                            nc.scalar.copy(ot[:], wp[0:128, 0:512])
                            nc.sync.dma_start(
                                out=outT[128 * ob:128 * (ob + 1),
                                         c0:c0 + 512],
                                in_=ot[:])

                    yt_pref = []
                    for Q in range(NSB):
                        ssB = ps_ss(f"ssB{Q}")
                        pend = []
                        for h in range(NH_LOC):
                            pend.append(attn_scores(h, Q, ssB))
                            if len(pend) >= 3:
                                attn_pv(pend.pop(0))
                        while pend:
                            attn_pv(pend.pop(0))
                        nc.gpsimd.collective_compute(
                            "AllGather", mybir.AluOpType.bypass,
                            replica_groups=[list(range(N_CORES))],
                            ins=[agi[Q][:]], outs=[ago[Q][:]])
                        if Q >= 2:
                            yt_pref.append(wo_load(Q - 1))
                            wo_chunk(Q - 2, yt_pref.pop(0))
                        elif Q == 1:
                            yt_pref.append(wo_load(0))
                    yt_pref.append(wo_load(NSB - 1))
                    wo_chunk(NSB - 2, yt_pref.pop(0))
                    wo_chunk(NSB - 1, yt_pref.pop(0))

    nc.compile()
    return nc


_PERM = np.concatenate([np.arange(0, HD, 2), np.arange(1, HD, 2)])

# gathered-y row order is h-major: row 512h + 64c + d holds global channel
# 256c + 64h + d; permute wo's input dims to match
_CI_PERM = np.empty(C, np.int64)
for _h in range(NH_LOC):
    for _c in range(N_CORES):
        _CI_PERM[512 * _h + 64 * _c:512 * _h + 64 * _c + 64] = \
            np.arange(256 * _c + 64 * _h, 256 * _c + 64 * _h + 64)


def _prep_inputs(x, freqs_cos, freqs_sin, wq, wk, wv, wo):
    from ml_dtypes import bfloat16
    x = np.asarray(x, np.float32)
    wq = np.asarray(wq, np.float32)
    wk = np.asarray(wk, np.float32)
    wv = np.asarray(wv, np.float32)
    wo = np.asarray(wo, np.float32)
    fc = np.asarray(freqs_cos, np.float32).T   # [32, T]
    fs = np.asarray(freqs_sin, np.float32).T

    xT = np.ascontiguousarray(x[0].T).astype(bfloat16)          # [C, T]
    cs128 = np.ascontiguousarray(np.concatenate([fc, fc, fc, fc], axis=0))
    sn128 = np.ascontiguousarray(np.concatenate([-fs, fs, -fs, fs], axis=0))

    in_maps = []
    for c in range(N_CORES):
        wq_c = wq[QD * c:QD * (c + 1), :].reshape(NH_LOC, HD, C)
        wq_c = wq_c[:, _PERM, :].reshape(QD, C)
        wk_c = wk[HD * c:HD * (c + 1), :][_PERM, :]
        wv_c = wv[HD * c:HD * (c + 1), :]
        wqkv = np.concatenate([wq_c.T, wk_c.T, wv_c.T], axis=1)
        in_maps.append({
            "xT": xT,
            "wqkvT": np.ascontiguousarray(wqkv).astype(bfloat16),
            "woT": np.ascontiguousarray(
                wo[QD * c:QD * (c + 1), :].T).astype(bfloat16),
            "cs128": cs128,
            "sn128": sn128,
        })
    return in_maps


def get_nc():
    if "nc" not in _CACHE:
        _CACHE["nc"] = _build()
    return _CACHE["nc"]


def kernel(x, freqs_cos, freqs_sin, wq, wk, wv, wo, **run_kwargs):
    from concourse.bass_utils import run_bass_kernel_spmd
    nc = get_nc()
    in_maps = _prep_inputs(x, freqs_cos, freqs_sin, wq, wk, wv, wo)
    res = run_bass_kernel_spmd(nc, in_maps, list(range(N_CORES)), **run_kwargs)
    outT = np.concatenate([res.results[c]["outT"] for c in range(N_CORES)],
                          axis=0)
    out = np.ascontiguousarray(outT.T).reshape(1, T, C).astype(np.float32)
    if run_kwargs:
        kernel.last_results = res
    return out


# revision 21
# speedup vs baseline: 1.1563x; 1.1563x over previous
"""GQA sliding-window attention (B=1, T=2048, C=2048, 32 Q / 8 KV heads,
head_dim=64, window=512, 16 global tokens) on 8 Trainium2 NeuronCores.

Sharding: tensor-parallel over heads — core c owns KV head c and Q heads
4c..4c+3.  Everything on-device runs transposed ([feature, token] layout).

v2 layout: x/weights/gathered-y all bf16 (rel-err budget 2e-2, measured
~1e-3); per-(head,chunk) scores are packed into one 5-bank PSUM strip
(cols 0-1279 = window lower-edge blocks, 1280-2559 = causal diag blocks)
so softmax exp is 2 big activations instead of 8 small ones; 1/den via
vector reciprocal (no Exp/Ln act-table thrash); masks split between
gpsimd affine_select (edge) and vector mul (diag); PV lags scores by one
head so the PE never waits on exp.  yT is AllGather'd per 512-token
chunk in bf16 (overlapped with the next chunk's attention); WO runs two
chunks behind.
"""

import sys
sys.path.insert(0, "/opt/trn_rl_repo")

import numpy as np

import concourse.bass as bass
import concourse.mybir as mybir
from concourse import bacc
from concourse.tile import TileContext
from concourse.masks import make_identity

f32 = mybir.dt.float32
f32r = mybir.dt.float32r
bf16 = mybir.dt.bfloat16
AF = mybir.ActivationFunctionType

N_CORES = 8
T = 2048
C = 2048
HD = 64
NH_LOC = 4            # query heads per core
QD = NH_LOC * HD      # 256 per-core query dims
NB = T // 128         # 16 token blocks
NSB = T // 512        # 4 superblocks
N_GLOBAL = 16
SCALE = 0.125         # 1/sqrt(64)

_CACHE = {}


def _segs(Q, which):
    """Score-strip segment tables: (j, col0, qlo, nblk).

    strip1 ([128,1024], 2 banks): 3 of the window lower-edge k-blocks j<4Q.
    strip2 ([128,1536], 3 banks): the 4th edge block + causal k-blocks
    j>=4Q.  Edge segs mask their (j+4)-qlo block, diag segs their first.
    Column packing keeps every matmul region inside one 2KB psum bank.
    """
    if which == 1:
        if Q == 0:
            return []
        return [(4 * Q - 1, 0, 4 * Q, 4), (4 * Q - 2, 512, 4 * Q, 3),
                (4 * Q - 4, 896, 4 * Q, 1)]
    s = [] if Q == 0 else [(4 * Q - 3, 0, 4 * Q, 2)]
    return s + [(4 * Q + 2, 256, 4 * Q + 2, 2), (4 * Q, 512, 4 * Q, 4),
                (4 * Q + 1, 1024, 4 * Q + 1, 3), (4 * Q + 3, 1408,
                                                  4 * Q + 3, 1)]


def _build():
    nc = bacc.Bacc(num_devices=N_CORES)

    xT = nc.declare_dram_parameter("xT", [C, T], bf16, isOutput=False)
    wqkvT = nc.declare_dram_parameter("wqkvT", [C, QD + 128], bf16,
                                      isOutput=False)
    woT = nc.declare_dram_parameter("woT", [C, QD], bf16, isOutput=False)
    cs1 = nc.declare_dram_parameter("cs128", [128, T], f32, isOutput=False)
    sn1 = nc.declare_dram_parameter("sn128", [128, T], f32, isOutput=False)
    outT = nc.declare_dram_parameter("outT", [QD, T], f32, isOutput=True)

    with TileContext(nc) as tc:
        with tc.tile_pool(name="persist", bufs=1) as pp, \
             tc.tile_pool(name="psAll", bufs=1, space="PSUM") as psA, \
             tc.tile_pool(name="pdram", bufs=1, space="DRAM") as pdr:
            # ---- persistent state ----
            qTr01 = pp.tile([128, T], f32r)   # rotated q, heads 0,1
            qTr23 = pp.tile([128, T], f32r)   # rotated q, heads 2,3
            kTr2 = pp.tile([128, T], f32r)    # rotated k duplicated on halves
            vT = pp.tile([64, T], f32)        # v (d, t) pre-transpose
            vgs = [pp.tile([128, HD + 1], f32r, name=f"vg{j}", tag=f"vg{j}")
                   for j in range(NB)]
            ytn = [pp.tile([64, T], bf16, name=f"ytn{h}", tag=f"ytn{h}")
                   for h in range(NH_LOC)]
            csb = pp.tile([128, T], f32)
            snb = pp.tile([128, T], f32)
            ident = pp.tile([128, 128], f32)
            m_diag = pp.tile([128, 128], f32)

            make_identity(nc, ident[:])
            # m_diag keeps p <= f (causal; scoresT layout: partition=key,
            # free=query)
            nc.gpsimd.memset(m_diag[:], 1.0)
            nc.gpsimd.affine_select(out=m_diag[:], in_=m_diag[:],
                                    compare_op=mybir.AluOpType.is_ge, fill=0.0,
                                    base=0, pattern=[[1, 128]],
                                    channel_multiplier=-1)
            ones1 = pp.tile([128, 1], f32)
            nc.vector.memset(ones1[:], 1.0)
            for j in range(NB):
                nc.vector.tensor_copy(vgs[j][:, HD:HD + 1], ones1[:])

            # tiny warmup AllGather so the first real one doesn't pay the
            # collective-stream cold start; overlaps the QKV phase
            agw_i = pdr.tile([1, 64], f32, name="agw_i", tag="agw_i")
            agw_o = pdr.tile([N_CORES, 64], f32, name="agw_o", tag="agw_o",
                             addr_space="Shared")
            nc.gpsimd.collective_compute(
                "AllGather", mybir.AluOpType.bypass,
                replica_groups=[list(range(N_CORES))],
                ins=[agw_i[:]], outs=[agw_o[:]])

            # psum tags (8 banks): s1 (2) + s2 (3) score strips — separate
            # tiles so their pipelines don't false-share WAR deps; yq (2) =
            # PV accumulator / WO psum / phase-A q01; aux (1) = V-transpose
            # + global-score scratch.  Phase A q23/kv alternate s1/s2.
            def ps_s1(name, cols=1024):
                return psA.tile([128, cols], f32, tag="s1", name=name,
                                padded_shape=[128, 1024])

            def ps_s2(name, cols=1536):
                return psA.tile([128, cols], f32, tag="s2", name=name,
                                padded_shape=[128, 1536])

            def ps_yq(name):
                return psA.tile([128, 512], f32, tag="yq", bufs=2, name=name,
                                padded_shape=[128, 512])

            def ps_aux(name):
                return psA.tile([128, 512], f32, tag="aux", bufs=1, name=name,
                                padded_shape=[128, 512])

            with tc.tile_pool(name="pbc", bufs=1) as pbc:
                # ============= phase A: QKV projections + RoPE =============
                with tc.tile_pool(name="pa", bufs=1) as pa:
                    wqkv_sb = [pa.tile([128, QD + 128], bf16, name=f"wqkv{k}",
                                       tag=f"wqkv{k}") for k in range(16)]
                    xt_sb = [pa.tile([128, T], bf16, name=f"xt{k}",
                                     tag=f"xt{k}") for k in range(16)]
                    for kt in range(16):
                        nc.sync.dma_start(
                            out=wqkv_sb[kt][:],
                            in_=wqkvT[128 * kt:128 * (kt + 1), :])
                        nc.sync.dma_start(
                            out=xt_sb[kt][:, 0:1024],
                            in_=xT[128 * kt:128 * (kt + 1), 0:1024])
                        if kt == 4:
                            nc.sync.dma_start(out=csb[:], in_=cs1[:])
                            nc.sync.dma_start(out=snb[:], in_=sn1[:])
                    for kt in range(16):
                        nc.sync.dma_start(
                            out=xt_sb[kt][:, 1024:2048],
                            in_=xT[128 * kt:128 * (kt + 1), 1024:2048])

                    def rope(dst, psrc, rows, col0):
                        """dst[0:rows, col0:+512] = rope(psrc); head dims are
                        pre-permuted to rotate-half order (evens then odds).
                        Swap-copies split across scalar+vector so neither
                        engine paces the chunk."""
                        qs = pa.tile([rows, 512], f32, tag="qs", bufs=2)
                        for b in range(rows // 32):
                            s = b ^ 1
                            eng = nc.scalar.copy if b % 2 == 0 else \
                                nc.vector.tensor_copy
                            eng(qs[32 * b:32 * b + 32, :],
                                psrc[32 * s:32 * s + 32, :])
                        t1 = pa.tile([rows, 512], f32, tag="t1", bufs=2)
                        t2 = pa.tile([rows, 512], f32, tag="t2", bufs=2)
                        nc.vector.tensor_mul(t1[:], psrc,
                                             csb[0:rows, col0:col0 + 512])
                        nc.vector.tensor_mul(t2[:], qs[:],
                                             snb[0:rows, col0:col0 + 512])
                        nc.vector.tensor_add(dst[0:rows, col0:col0 + 512],
                                             t1[:], t2[:])

                    for tcc in range(NSB):
                        c0 = 512 * tcc
                        q01t = ps_yq(f"q01_{tcc}")
                        q01 = q01t[0:128, 0:512]
                        if tcc % 2 == 0:
                            q23t = ps_s1(f"q23_{tcc}", 512)
                            kvt = ps_s2(f"kv{tcc}", 512)
                        else:
                            q23t = ps_s2(f"q23_{tcc}", 512)
                            kvt = ps_s1(f"kv{tcc}", 512)
                        q23 = q23t[0:128, 0:512]
                        kv = kvt[0:128, 0:512]
                        kv_k = kvt[0:64, 0:512]
                        kv_v = kvt[64:128, 0:512]
                        # region-sequential matmuls so each region's rope
                        # overlaps the next region's matmuls
                        for kt in range(16):
                            nc.tensor.matmul(q01, wqkv_sb[kt][:, 0:128],
                                             xt_sb[kt][:, c0:c0 + 512],
                                             start=kt == 0, stop=kt == 15)
                        rope(qTr01, q01, 128, c0)
                        for kt in range(16):
                            nc.tensor.matmul(q23, wqkv_sb[kt][:, 128:256],
                                             xt_sb[kt][:, c0:c0 + 512],
                                             start=kt == 0, stop=kt == 15)
                        rope(qTr23, q23, 128, c0)
                        for kt in range(16):
                            nc.tensor.matmul(kv, wqkv_sb[kt][:, 256:384],
                                             xt_sb[kt][:, c0:c0 + 512],
                                             start=kt == 0, stop=kt == 15)
                        rope(kTr2, kv_k, 64, c0)
                        for b in range(2):
                            nc.vector.tensor_copy(
                                kTr2[64 + 32 * b:96 + 32 * b, c0:c0 + 512],
                                kTr2[32 * b:32 * (b + 1), c0:c0 + 512])
                        nc.scalar.copy(vT[:, c0:c0 + 512], kv_v)
                        for j in range(4 * tcc, 4 * tcc + 4):
                            ptr = ps_aux(f"ptr{j}")
                            nc.tensor.transpose(ptr[0:128, 0:64],
                                                vT[:, 128 * j:128 * (j + 1)],
                                                ident[0:64, 0:64])
                            nc.scalar.copy(vgs[j][:, 0:HD], ptr[0:128, 0:64])

                # ===== attention (Q-outer) + chunked AllGather + lagged WO ==
                with tc.tile_pool(name="pc", bufs=1) as pc:
                    wo_sb = [pc.tile([128, QD], bf16, name=f"wo{k}",
                                     tag=f"wo{k}") for k in range(16)]
                    for k in range(16):
                        nc.sync.dma_start(out=wo_sb[k][:],
                                          in_=woT[128 * k:128 * (k + 1), :])
                    agi = [pdr.tile([QD, 512], bf16, name=f"agi{Q}",
                                    tag=f"agi{Q}") for Q in range(NSB)]
                    ago = [pdr.tile([C, 512], bf16, name=f"ago{Q}",
                                    tag=f"ago{Q}", addr_space="Shared")
                           for Q in range(NSB)]

                    def _mask_seg(Q, pt, j, a, qlo, nblk):
                        if j >= 4 * Q:     # causal diagonal block (first)
                            nc.vector.tensor_mul(pt[:, a:a + 128],
                                                 pt[:, a:a + 128], m_diag[:])
                        else:              # window lower-edge block
                            e = a + 128 * (j + 4 - qlo)
                            nc.gpsimd.affine_select(
                                out=pt[:, e:e + 128], in_=pt[:, e:e + 128],
                                compare_op=mybir.AluOpType.is_ge,
                                fill=0.0, base=-1, pattern=[[-1, 128]],
                                channel_multiplier=1)
                        # kill window copies of global keys in k-block 0
                        if j == 0:
                            nc.gpsimd.affine_select(
                                out=pt[:, a:a + 128 * nblk],
                                in_=pt[:, a:a + 128 * nblk],
                                compare_op=mybir.AluOpType.is_ge,
                                fill=0.0, base=-N_GLOBAL,
                                pattern=[[0, 128 * nblk]],
                                channel_multiplier=1)

                    def attn_scores(h, Q):
                        """Emit score matmuls + exps + masks for head h of
                        chunk Q into per-head psum strips; returns the state
                        the (lagged) PV stage needs."""
                        qt = qTr01 if h < 2 else qTr23
                        qb = 64 * (h % 2)
                        c0 = 512 * Q
                        sg = ps_aux(f"sg{Q}_{h}")
                        nc.tensor.matmul(sg[0:N_GLOBAL, 0:512],
                                         kTr2[qb:qb + 64, 0:N_GLOBAL],
                                         qt[qb:qb + 64, c0:c0 + 512],
                                         start=True, stop=True)
                        pg = pbc.tile([N_GLOBAL, 512], f32r, tag="pg", bufs=3,
                                      name=f"pg{Q}_{h}")
                        nc.scalar.activation(pg[:], sg[0:N_GLOBAL, 0:512],
                                             AF.Exp, scale=SCALE)
                        s1, s2 = _segs(Q, 1), _segs(Q, 2)
                        pt1 = None
                        if s1:
                            st1 = ps_s1(f"st1_{Q}_{h}")
                            pt1 = pbc.tile([128, 1024], f32r, tag="pt1",
                                           bufs=3, name=f"pt1_{Q}_{h}")
                            for j, a, qlo, nblk in s1:
                                nc.tensor.matmul(
                                    st1[0:128, a:a + 128 * nblk],
                                    kTr2[qb:qb + 64, 128 * j:128 * (j + 1)],
                                    qt[qb:qb + 64,
                                       128 * qlo:128 * qlo + 128 * nblk],
                                    start=True, stop=True)
                            nc.scalar.activation(pt1[:], st1[0:128, 0:1024],
                                                 AF.Exp, scale=SCALE)
                            for seg in s1:
                                _mask_seg(Q, pt1, *seg)
                        st2 = ps_s2(f"st2_{Q}_{h}")
                        pt2 = pbc.tile([128, 1536], f32r, tag="pt2", bufs=3,
                                       name=f"pt2_{Q}_{h}")
                        for j, a, qlo, nblk in s2:
                            nc.tensor.matmul(
                                st2[0:128, a:a + 128 * nblk],
                                kTr2[qb:qb + 64, 128 * j:128 * (j + 1)],
                                qt[qb:qb + 64,
                                   128 * qlo:128 * qlo + 128 * nblk],
                                start=True, stop=True)
                        nc.scalar.activation(pt2[:], st2[0:128, 0:1536],
                                             AF.Exp, scale=SCALE)
                        for seg in s2:
                            _mask_seg(Q, pt2, *seg)
                        return (h, Q, pt1, pt2, pg)

                    def attn_pv(state):
                        """Lagged PV + normalize + ship to the gather buf."""
                        h, Q, pt1, pt2, pg = state
                        c0 = 512 * Q
                        yq = ps_yq(f"yq{Q}_{h}")
                        nc.tensor.matmul(yq[0:HD + 1, 0:512],
                                         vgs[0][0:N_GLOBAL, :], pg[:],
                                         start=True, stop=False)
                        segs = [(pt1, s) for s in _segs(Q, 1)] + \
                               [(pt2, s) for s in _segs(Q, 2)]
                        for idx, (pt, (j, a, qlo, nblk)) in enumerate(segs):
                            o = 128 * (qlo - 4 * Q)
                            nc.tensor.matmul(
                                yq[0:HD + 1, o:o + 128 * nblk],
                                vgs[j][:], pt[:, a:a + 128 * nblk],
                                start=False, stop=(idx == len(segs) - 1))
                        dens = pbc.tile([1, 512], f32, tag="dens", bufs=2,
                                        name=f"dens{Q}_{h}")
                        nc.vector.tensor_copy(dens[:], yq[HD:HD + 1, 0:512])
                        denr = pbc.tile([1, 512], f32, tag="denr", bufs=2,
                                        name=f"denr{Q}_{h}")
                        nc.vector.reciprocal_approx_fast(denr[:], dens[:])
                        rb = pbc.tile([64, 512], f32, tag="rb", bufs=2,
                                      name=f"rb{Q}_{h}")
                        nc.gpsimd.partition_broadcast(rb[:], denr[:])
                        nc.vector.tensor_mul(ytn[h][:, c0:c0 + 512],
                                             yq[0:HD, 0:512], rb[:])
                        nc.sync.dma_start(
                            out=agi[Q][64 * h:64 * (h + 1), :],
                            in_=ytn[h][:, c0:c0 + 512])

                    def wo_load(Q):
                        yt = pc.tile([128, 16 * 512], bf16, tag="yt", bufs=2,
                                     name=f"yt{Q}")
                        # yt[p, 512*ci + c] = ago[Q][128*ci + p, c]
                        nc.sync.dma_start(
                            out=yt.rearrange("p (ci c) -> p ci c", c=512),
                            in_=ago[Q].rearrange("(ci p) c -> p ci c", p=128))
                        return yt

                    def wo_chunk(Q, yt):
                        c0 = 512 * Q
                        for ob in range(2):
                            wp = ps_yq(f"wp{Q}_{ob}")
                            for ci in range(16):
                                nc.tensor.matmul(
                                    wp[0:128, 0:512],
                                    wo_sb[ci][:, 128 * ob:128 * (ob + 1)],
                                    yt[:, 512 * ci:512 * (ci + 1)],
                                    start=(ci == 0), stop=(ci == 15))
                            ot = pc.tile([128, 512], f32, tag="ot", bufs=2,
                                         name=f"ot{Q}_{ob}")
                            nc.scalar.copy(ot[:], wp[0:128, 0:512])
                            nc.sync.dma_start(
                                out=outT[128 * ob:128 * (ob + 1),
                                         c0:c0 + 512],
                                in_=ot[:])

                    yt_pref = []
                    for Q in range(NSB):
                        pend = []
                        for h in range(NH_LOC):
                            pend.append(attn_scores(h, Q))
                            if len(pend) >= 3:
                                attn_pv(pend.pop(0))
                        while pend:
                            attn_pv(pend.pop(0))
                        nc.gpsimd.collective_compute(
                            "AllGather", mybir.AluOpType.bypass,
                            replica_groups=[list(range(N_CORES))],
                            ins=[agi[Q][:]], outs=[ago[Q][:]])
                        if Q >= 2:
                            yt_pref.append(wo_load(Q - 1))
                            wo_chunk(Q - 2, yt_pref.pop(0))
                        elif Q == 1:
                            yt_pref.append(wo_load(0))
                    yt_pref.append(wo_load(NSB - 1))
                    wo_chunk(NSB - 2, yt_pref.pop(0))
                    wo_chunk(NSB - 1, yt_pref.pop(0))

    nc.compile()
    return nc


_PERM = np.concatenate([np.arange(0, HD, 2), np.arange(1, HD, 2)])

# gathered-y row order is h-major: row 512h + 64c + d holds global channel
# 256c + 64h + d; permute wo's input dims to match
_CI_PERM = np.empty(C, np.int64)
for _h in range(NH_LOC):
    for _c in range(N_CORES):
        _CI_PERM[512 * _h + 64 * _c:512 * _h + 64 * _c + 64] = \
            np.arange(256 * _c + 64 * _h, 256 * _c + 64 * _h + 64)


def _prep_inputs(x, freqs_cos, freqs_sin, wq, wk, wv, wo):
    from ml_dtypes import bfloat16
    x = np.asarray(x, np.float32)
    wq = np.asarray(wq, np.float32)
    wk = np.asarray(wk, np.float32)
    wv = np.asarray(wv, np.float32)
    wo = np.asarray(wo, np.float32)
    fc = np.asarray(freqs_cos, np.float32).T   # [32, T]
    fs = np.asarray(freqs_sin, np.float32).T

    xT = np.ascontiguousarray(x[0].T).astype(bfloat16)          # [C, T]
    cs128 = np.ascontiguousarray(np.concatenate([fc, fc, fc, fc], axis=0))
    sn128 = np.ascontiguousarray(np.concatenate([-fs, fs, -fs, fs], axis=0))

    in_maps = []
    for c in range(N_CORES):
        wq_c = wq[QD * c:QD * (c + 1), :].reshape(NH_LOC, HD, C)
        wq_c = wq_c[:, _PERM, :].reshape(QD, C)
        wk_c = wk[HD * c:HD * (c + 1), :][_PERM, :]
        wv_c = wv[HD * c:HD * (c + 1), :]
        wqkv = np.concatenate([wq_c.T, wk_c.T, wv_c.T], axis=1)
        in_maps.append({
            "xT": xT,
            "wqkvT": np.ascontiguousarray(wqkv).astype(bfloat16),
            "woT": np.ascontiguousarray(
                wo[QD * c:QD * (c + 1), :].T).astype(bfloat16),
            "cs128": cs128,
            "sn128": sn128,
        })
    return in_maps


def get_nc():
    if "nc" not in _CACHE:
        _CACHE["nc"] = _build()
    return _CACHE["nc"]


def kernel(x, freqs_cos, freqs_sin, wq, wk, wv, wo, **run_kwargs):
    from concourse.bass_utils import run_bass_kernel_spmd
    nc = get_nc()
    in_maps = _prep_inputs(x, freqs_cos, freqs_sin, wq, wk, wv, wo)
    res = run_bass_kernel_spmd(nc, in_maps, list(range(N_CORES)), **run_kwargs)
    outT = np.concatenate([res.results[c]["outT"] for c in range(N_CORES)],
                          axis=0)
    out = np.ascontiguousarray(outT.T).reshape(1, T, C).astype(np.float32)
    if run_kwargs:
        kernel.last_results = res
    return out
